# revision 14
# baseline (speedup 1.0000x reference)
"""Trainium2 Bass kernel for nn_ModelNew_1580547973039.

Math (see the collapsed reference):
    y0 = (sum_hw x) @ (sum_kk W) / (Hout*Wout) + bias      # [N, Cout]
    y1 = clip(LN(y0)*g1 + b1, -1, 1)
    y2 = LN(y1)*g2 + b2

Sharding: data-parallel over N across 8 cores (32 rows each); params
(weight sums, bias, LN gammas/betas) replicated.  The only heavy work is
streaming x (512 MB total, 64 MB/core) and reducing over the 4096 spatial
positions — 32 x [128c, 4096hw] tiles per core, HWDGE DMA double-buffered
over NBUF slots, DVE reduce_sum per tile.  wsum = weight.sum((2,3))/denom
is precomputed on host (4 MB of weight, trivial) and replicated, per the
sharding hint.

Raw Bass (no Tile): this walrus build rejects DMACopy instructions with
more than one sync-wait (one TPB_EVENTS slot), which Tile's scheduler
emits for any slot-recycled streaming loop.  With explicit semaphores the
waits sit on the issuing engine's sequencer instead.
"""

import sys
from contextlib import ExitStack

import numpy as np

if "/opt/trn_rl_repo" not in sys.path:
    sys.path.insert(0, "/opt/trn_rl_repo")

import concourse.bass as bass
from concourse import mybir
from concourse.bass_utils import run_bass_kernel_spmd

# Problem shapes (hardcoded; kernel.py must be self-contained).
N, CIN, H, W = 256, 128, 64, 64
COUT = 512
HW = H * W
NCORES = 8
NPC = N // NCORES  # rows of N per core
EPS = 1e-5
STRIDE, KSIZE = 2, 4
HOUT = (H - 1) * STRIDE + KSIZE
WOUT = (W - 1) * STRIDE + KSIZE
DENOM = float(HOUT * WOUT)

NBUF = 8  # x-tile double-buffer depth

F32 = mybir.dt.float32
AX = mybir.AxisListType
ALU = mybir.AluOpType
ACTF = mybir.ActivationFunctionType

_CACHED_NC = None


def _build_bass():
    """Per-core SPMD program: x shard [NPC, CIN, HW] -> out [NPC, COUT]."""
    nc = bass.Bass()

    x_h = nc.declare_dram_parameter("x", [NPC, CIN, HW], F32, isOutput=False)
    wsum_h = nc.declare_dram_parameter("wsum", [CIN, COUT], F32, isOutput=False)
    bias_h = nc.declare_dram_parameter("bias", [COUT], F32, isOutput=False)
    g1_h = nc.declare_dram_parameter("g1", [COUT], F32, isOutput=False)
    b1_h = nc.declare_dram_parameter("b1", [COUT], F32, isOutput=False)
    g2_h = nc.declare_dram_parameter("g2", [COUT], F32, isOutput=False)
    b2_h = nc.declare_dram_parameter("b2", [COUT], F32, isOutput=False)
    out_h = nc.declare_dram_parameter("out", [NPC, COUT], F32, isOutput=True)

    with ExitStack() as ctx:
        sb = lambda name, shape: ctx.enter_context(nc.sbuf_tensor(name, shape, F32))
        xt = [sb(f"xt{b}", [CIN, HW]) for b in range(NBUF)]
        wsum_t = sb("wsum_t", [CIN, COUT])
        bias_t = sb("bias_t", [1, COUT])
        ones_t = sb("ones_t", [1, NPC])
        eps_t = sb("eps_t", [NPC, 1])
        g1_t = sb("g1_t", [NPC, COUT])
        b1_t = sb("b1_t", [NPC, COUT])
        g2_t = sb("g2_t", [NPC, COUT])
        b2_t = sb("b2_t", [NPC, COUT])
        xs_t = sb("xs_t", [CIN, NPC])
        y0_t = sb("y0_t", [NPC, COUT])
        y1_t = sb("y1_t", [NPC, COUT])
        y2_t = sb("y2_t", [NPC, COUT])
        sq_t = sb("sq_t", [NPC, COUT])
        mu1_t = sb("mu1_t", [NPC, 1])
        mu2_t = sb("mu2_t", [NPC, 1])
        vs1_t = sb("vs1_t", [NPC, 1])
        vs2_t = sb("vs2_t", [NPC, 1])
        sd1_t = sb("sd1_t", [NPC, 1])
        sd2_t = sb("sd2_t", [NPC, 1])
        ps = ctx.enter_context(nc.psum_tensor("ps", [NPC, COUT], F32))

        sem_x = [ctx.enter_context(nc.semaphore(f"sx{b}")) for b in range(NBUF)]
        sem_const = ctx.enter_context(nc.semaphore("sconst"))
        sem_red = ctx.enter_context(nc.semaphore("sred"))
        sem_mm = ctx.enter_context(nc.semaphore("smm"))
        sem_v1 = ctx.enter_context(nc.semaphore("sv1"))
        sem_a1 = ctx.enter_context(nc.semaphore("sa1"))
        sem_v2 = ctx.enter_context(nc.semaphore("sv2"))
        sem_a2 = ctx.enter_context(nc.semaphore("sa2"))
        sem_sq = ctx.enter_context(nc.semaphore("ssq"))
        sem_e = ctx.enter_context(nc.semaphore("se"))
        sem_ms = ctx.enter_context(nc.semaphore("sms"))
        sem_y2 = ctx.enter_context(nc.semaphore("sy2"))
        sem_out = ctx.enter_context(nc.semaphore("sout"))
        block = ctx.enter_context(nc.Block())

        @block.sync
        def _(sync):
            # Replicated params (6 transfers -> sem_const reaches 96).
            sync.dma_start(out=wsum_t[:], in_=wsum_h[:]).then_inc(sem_const, 16)
            sync.dma_start(out=bias_t[:], in_=bias_h[:][None, :]).then_inc(
                sem_const, 16
            )
            for dst, src in ((g1_t, g1_h), (b1_t, b1_h), (g2_t, g2_h), (b2_t, b2_h)):
                sync.dma_start(
                    out=dst[:], in_=src[:][None, :].to_broadcast((NPC, COUT))
                ).then_inc(sem_const, 16)
            # Stream x tiles.
            for i in range(NPC):
                b = i % NBUF
                if i >= NBUF:
                    # Slot reuse: wait until the reduce of the previous tenant
                    # is done (implies its DMA landed too).
                    sync.wait_ge(sem_red, i - NBUF + 1)
                sync.dma_start(out=xt[b][:], in_=x_h[i]).then_inc(sem_x[b], 16)
            # Write result out once the epilogue finishes.
            sync.wait_ge(sem_y2, 1)
            sync.dma_start(out=out_h[:], in_=y2_t[:]).then_inc(sem_out, 16)
            sync.wait_ge(sem_out, 16)

        @block.vector
        def _(vector):
            # Raw-bass rule on this stack: an engine's instructions issue
            # pipelined with NO same-engine ordering of data effects.  Every
            # same-engine RAW/WAR edge below gets an explicit sem hop.
            ev = 0  # sem_e target tracker

            def hop(inst):
                nonlocal ev
                ev += 1
                inst.then_inc(sem_e, 1)
                vector.wait_ge(sem_e, ev)

            vector.memset(ones_t[:], 1.0).then_inc(sem_ms, 1)
            vector.memset(eps_t[:], EPS).then_inc(sem_ms, 1)
            # Spatial reduction: xs[c, n] = sum_hw x[n, c, hw].  The reduces
            # are mutually independent (distinct xs columns / xt slots).
            for i in range(NPC):
                b = i % NBUF
                vector.wait_ge(sem_x[b], 16 * (i // NBUF + 1))
                nc.vector.reduce_sum(
                    xs_t[:, i : i + 1], xt[b][:], axis=AX.X
                ).then_inc(sem_red, 1)
            # Epilogue: LN -> hardtanh -> LN on [NPC, COUT].
            vector.wait_ge(sem_mm, 1)
            hop(nc.vector.tensor_copy(y0_t[:], ps[:]))
            hop(nc.vector.reduce_sum(mu1_t[:], y0_t[:], axis=AX.X))
            hop(nc.vector.tensor_scalar_mul(mu1_t[:], in0=mu1_t[:], scalar1=-1.0 / COUT))
            # y1 = y0 - mean
            nc.vector.tensor_scalar_add(y1_t[:], in0=y0_t[:], scalar1=mu1_t[:]).then_inc(
                sem_v1, 1
            )
            vector.wait_ge(sem_a1, 1)
            hop(nc.vector.reciprocal(out=sd1_t[:], in_=sd1_t[:]))
            # y1 = (y0 - mean) * rstd * g1 + b1, then hardtanh
            hop(nc.vector.tensor_scalar_mul(y1_t[:], in0=y1_t[:], scalar1=sd1_t[:]))
            vector.wait_ge(sem_const, 96)
            hop(nc.vector.tensor_mul(y1_t[:], y1_t[:], g1_t[:]))
            hop(nc.vector.tensor_add(y1_t[:], y1_t[:], b1_t[:]))
            hop(
                nc.vector.tensor_scalar(
                    out=y1_t[:],
                    in0=y1_t[:],
                    scalar1=-1.0,
                    scalar2=1.0,
                    op0=ALU.max,
                    op1=ALU.min,
                )
            )
            hop(nc.vector.reduce_sum(mu2_t[:], y1_t[:], axis=AX.X))
            hop(nc.vector.tensor_scalar_mul(mu2_t[:], in0=mu2_t[:], scalar1=-1.0 / COUT))
            nc.vector.tensor_scalar_add(y2_t[:], in0=y1_t[:], scalar1=mu2_t[:]).then_inc(
                sem_v2, 1
            )
            vector.wait_ge(sem_a2, 1)
            hop(nc.vector.reciprocal(out=sd2_t[:], in_=sd2_t[:]))
            hop(nc.vector.tensor_scalar_mul(y2_t[:], in0=y2_t[:], scalar1=sd2_t[:]))
            hop(nc.vector.tensor_mul(y2_t[:], y2_t[:], g2_t[:]))
            nc.vector.tensor_add(y2_t[:], y2_t[:], b2_t[:]).then_inc(sem_y2, 1)

        @block.tensor
        def _(tensor):
            tensor.wait_ge(sem_red, NPC)
            tensor.wait_ge(sem_const, 96)
            tensor.wait_ge(sem_ms, 2)
            nc.tensor.matmul(ps[:], lhsT=xs_t[:], rhs=wsum_t[:], start=True, stop=False)
            nc.tensor.matmul(
                ps[:], lhsT=ones_t[:], rhs=bias_t[:], start=False, stop=True
            ).then_inc(sem_mm, 1)

        @block.scalar
        def _(scalar):
            # vs = sum((y-mu)^2) via Square+accum; sd = sqrt(vs/COUT + eps).
            # sq_t is a discarded full-size activation output.
            scalar.wait_ge(sem_ms, 2)
            scalar.wait_ge(sem_v1, 1)
            nc.scalar.activation(
                sq_t[:], y1_t[:], ACTF.Square, accum_out=vs1_t[:]
            ).then_inc(sem_sq, 1)
            scalar.wait_ge(sem_sq, 1)
            nc.scalar.activation(
                sd1_t[:], vs1_t[:], ACTF.Sqrt, bias=eps_t[:], scale=1.0 / COUT
            ).then_inc(sem_a1, 1)
            scalar.wait_ge(sem_v2, 1)
            nc.scalar.activation(
                sq_t[:], y2_t[:], ACTF.Square, accum_out=vs2_t[:]
            ).then_inc(sem_sq, 1)
            scalar.wait_ge(sem_sq, 2)
            nc.scalar.activation(
                sd2_t[:], vs2_t[:], ACTF.Sqrt, bias=eps_t[:], scale=1.0 / COUT
            ).then_inc(sem_a2, 1)

    return nc


def _get_nc():
    global _CACHED_NC
    if _CACHED_NC is None:
        _CACHED_NC = _build_bass()
    return _CACHED_NC


def _run(inputs, **spmd_kwargs):
    x = np.ascontiguousarray(np.asarray(inputs["x"], dtype=np.float32))
    weight = np.asarray(inputs["weight"], dtype=np.float32)
    bias = np.ascontiguousarray(np.asarray(inputs["bias"], dtype=np.float32))
    g1 = np.ascontiguousarray(np.asarray(inputs["g1"], dtype=np.float32))
    b1 = np.ascontiguousarray(np.asarray(inputs["b1"], dtype=np.float32))
    g2 = np.ascontiguousarray(np.asarray(inputs["g2"], dtype=np.float32))
    b2 = np.ascontiguousarray(np.asarray(inputs["b2"], dtype=np.float32))

    wsum = np.ascontiguousarray(weight.sum(axis=(2, 3)) / DENOM)  # [CIN, COUT]
    x_sh = x.reshape(NCORES, NPC, CIN, HW)

    nc = _get_nc()
    core_ids = list(range(NCORES))
    in_maps = [
        {
            "x": np.ascontiguousarray(x_sh[i]),
            "wsum": wsum,
            "bias": bias,
            "g1": g1,
            "b1": b1,
            "g2": g2,
            "b2": b2,
        }
        for i in core_ids
    ]
    res = run_bass_kernel_spmd(nc, in_maps, core_ids, **spmd_kwargs)
    out = np.concatenate([res.results[i]["out"] for i in core_ids], axis=0)
    return out, res


def kernel(x, weight, bias, g1, b1, g2, b2):
    out, _ = _run(
        {"x": x, "weight": weight, "bias": bias, "g1": g1, "b1": b1, "g2": g2, "b2": b2}
    )
    return out


# revision 16
# speedup vs baseline: 1.0150x; 1.0150x over previous
"""Trainium2 Bass kernel for nn_ModelNew_1580547973039.

Math (see the collapsed reference):
    y0 = (sum_hw x) @ (sum_kk W) / (Hout*Wout) + bias      # [N, Cout]
    y1 = clip(LN(y0)*g1 + b1, -1, 1)
    y2 = LN(y1)*g2 + b2

Sharding: data-parallel over N across 8 cores (32 rows each); params
(weight sums, bias, LN gammas/betas) replicated.  The only heavy work is
streaming x (512 MB total, 64 MB/core) and reducing over the 4096 spatial
positions: [128c, nb*4096hw] tiles, HWDGE DMA double-buffered over NBUF
slots, DVE reduce_sum per tile.  wsum = weight.sum((2,3))/denom is
precomputed on host (4 MB of weight, trivial) and replicated, per the
sharding hint.

Raw Bass (no Tile): this walrus build rejects DMACopy instructions with
more than one sync-wait (one TPB_EVENTS slot), which Tile's scheduler
emits for any slot-recycled streaming loop.  With explicit semaphores the
waits sit on the issuing engine's sequencer instead.  IMPORTANT raw-bass
rule on this stack: engines give NO same-engine ordering of data effects
(DRAIN is a Tile-emitted instruction, absent here) — every same-engine
RAW/WAR edge needs an explicit semaphore hop.

Epilogue uses raw moments (var = E[y^2] - mu^2): ACT's Square+accum runs
concurrently with DVE's mean reduce, both directly from PSUM, then one
fused (y-mu)*rstd tensor_scalar.  When g1/b1/g2/b2 are identity (they are
for this problem's setup_inputs) the gamma/beta ops are compiled out; a
general fallback variant is built otherwise.
"""

import sys
from contextlib import ExitStack

import numpy as np

if "/opt/trn_rl_repo" not in sys.path:
    sys.path.insert(0, "/opt/trn_rl_repo")

import concourse.bass as bass
from concourse import mybir
from concourse.bass_utils import run_bass_kernel_spmd

# Problem shapes (hardcoded; kernel.py must be self-contained).
N, CIN, H, W = 256, 128, 64, 64
COUT = 512
HW = H * W
NCORES = 8
NPC = N // NCORES  # rows of N per core
EPS = 1e-5
STRIDE, KSIZE = 2, 4
HOUT = (H - 1) * STRIDE + KSIZE
WOUT = (W - 1) * STRIDE + KSIZE
DENOM = float(HOUT * WOUT)

# x streaming: 15 tiles of 2 rows (4 MiB) + 2 single-row tail tiles so the
# final reduce on the critical path is half-length.
TILE_ROWS = [2] * 15 + [1] * 2
assert sum(TILE_ROWS) == NPC
NT = len(TILE_ROWS)
NBUF = 5  # [128, 2*4096] f32 slots -> 160 KiB/partition

F32 = mybir.dt.float32
AX = mybir.AxisListType
ALU = mybir.AluOpType
ACTF = mybir.ActivationFunctionType

_CACHED_NC = {}


def _build_bass(apply_gb: bool):
    """Per-core SPMD program: x shard [NPC, CIN, HW] -> out [NPC, COUT]."""
    nc = bass.Bass()

    x_h = nc.declare_dram_parameter("x", [NPC, CIN, HW], F32, isOutput=False)
    wsum_h = nc.declare_dram_parameter("wsum", [CIN, COUT], F32, isOutput=False)
    bias_h = nc.declare_dram_parameter("bias", [COUT], F32, isOutput=False)
    if apply_gb:
        g1_h = nc.declare_dram_parameter("g1", [COUT], F32, isOutput=False)
        b1_h = nc.declare_dram_parameter("b1", [COUT], F32, isOutput=False)
        g2_h = nc.declare_dram_parameter("g2", [COUT], F32, isOutput=False)
        b2_h = nc.declare_dram_parameter("b2", [COUT], F32, isOutput=False)
    out_h = nc.declare_dram_parameter("out", [NPC, COUT], F32, isOutput=True)

    n_const = 96 if apply_gb else 32  # param DMAs x16

    with ExitStack() as ctx:
        sb = lambda name, shape: ctx.enter_context(nc.sbuf_tensor(name, shape, F32))
        xt = [sb(f"xt{b}", [CIN, 2, HW]) for b in range(NBUF)]
        wsum_t = sb("wsum_t", [CIN, COUT])
        bias_t = sb("bias_t", [1, COUT])
        ones_t = sb("ones_t", [1, NPC])
        eps_t = sb("eps_t", [NPC, 1])
        if apply_gb:
            g1_t = sb("g1_t", [NPC, COUT])
            b1_t = sb("b1_t", [NPC, COUT])
            g2_t = sb("g2_t", [NPC, COUT])
            b2_t = sb("b2_t", [NPC, COUT])
        xs_t = sb("xs_t", [CIN, NPC])
        y1_t = sb("y1_t", [NPC, COUT])
        y2_t = sb("y2_t", [NPC, COUT])
        sq_t = sb("sq_t", [NPC, COUT])  # discarded Square output
        mu1_t = sb("mu1_t", [NPC, 1])
        mu2_t = sb("mu2_t", [NPC, 1])
        msq1_t = sb("msq1_t", [NPC, 1])
        msq2_t = sb("msq2_t", [NPC, 1])
        vs1_t = sb("vs1_t", [NPC, 1])
        vs2_t = sb("vs2_t", [NPC, 1])
        t1_t = sb("t1_t", [NPC, 1])
        t2_t = sb("t2_t", [NPC, 1])
        sd1_t = sb("sd1_t", [NPC, 1])
        sd2_t = sb("sd2_t", [NPC, 1])
        ps = ctx.enter_context(nc.psum_tensor("ps", [NPC, COUT], F32))

        sem_x = [ctx.enter_context(nc.semaphore(f"sx{b}")) for b in range(NBUF)]
        sem_const = ctx.enter_context(nc.semaphore("sconst"))
        sem_red = ctx.enter_context(nc.semaphore("sred"))
        sem_mm = ctx.enter_context(nc.semaphore("smm"))
        sem_a = ctx.enter_context(nc.semaphore("sa"))  # ACT -> DVE data
        sem_d = ctx.enter_context(nc.semaphore("sd"))  # DVE -> ACT data
        sem_e = ctx.enter_context(nc.semaphore("se"))  # DVE self-order
        sem_ms = ctx.enter_context(nc.semaphore("sms"))  # memsets done
        sem_y2 = ctx.enter_context(nc.semaphore("sy2"))
        sem_out = ctx.enter_context(nc.semaphore("sout"))
        block = ctx.enter_context(nc.Block())

        row0 = []
        r = 0
        for nb in TILE_ROWS:
            row0.append(r)
            r += nb

        @block.sync
        def _(sync):
            # x stream only — params go on the scalar HWDGE queue.
            for i, nb in enumerate(TILE_ROWS):
                b = i % NBUF
                if i >= NBUF:
                    sync.wait_ge(sem_red, i - NBUF + 1)
                n0 = row0[i]
                sync.dma_start(
                    out=xt[b][:, :nb, :],
                    in_=x_h[n0 : n0 + nb].rearrange("n c s -> c n s"),
                ).then_inc(sem_x[b], 16)
            sync.wait_ge(sem_y2, 1)
            sync.dma_start(out=out_h[:], in_=y2_t[:]).then_inc(sem_out, 16)
            sync.wait_ge(sem_out, 16)

        @block.scalar
        def _(scalar):
            # Replicated params on the ACT HWDGE queue.
            nc.scalar.dma_start(out=wsum_t[:], in_=wsum_h[:]).then_inc(sem_const, 16)
            nc.scalar.dma_start(out=bias_t[:], in_=bias_h[:][None, :]).then_inc(
                sem_const, 16
            )
            if apply_gb:
                for dst, src in (
                    (g1_t, g1_h),
                    (b1_t, b1_h),
                    (g2_t, g2_h),
                    (b2_t, b2_h),
                ):
                    nc.scalar.dma_start(
                        out=dst[:], in_=src[:][None, :].to_broadcast((NPC, COUT))
                    ).then_inc(sem_const, 16)
            # Variance raw moments + sqrt, interleaved with DVE (see below).
            scalar.wait_ge(sem_ms, 2)
            scalar.wait_ge(sem_mm, 1)
            nc.scalar.activation(
                sq_t[:], ps[:], ACTF.Square, accum_out=vs1_t[:]
            ).then_inc(sem_a, 1)
            scalar.wait_ge(sem_d, 1)
            nc.scalar.activation(sd1_t[:], t1_t[:], ACTF.Sqrt, bias=eps_t[:]).then_inc(
                sem_a, 1
            )
            scalar.wait_ge(sem_d, 2)
            nc.scalar.activation(
                sq_t[:], y1_t[:], ACTF.Square, accum_out=vs2_t[:]
            ).then_inc(sem_a, 1)
            scalar.wait_ge(sem_d, 3)
            nc.scalar.activation(sd2_t[:], t2_t[:], ACTF.Sqrt, bias=eps_t[:]).then_inc(
                sem_a, 1
            )

        @block.vector
        def _(vector):
            ev = 0  # sem_e target tracker

            def hop(inst):
                nonlocal ev
                ev += 1
                inst.then_inc(sem_e, 1)
                vector.wait_ge(sem_e, ev)

            vector.memset(ones_t[:], 1.0).then_inc(sem_ms, 1)
            vector.memset(eps_t[:], EPS).then_inc(sem_ms, 1)
            # Spatial reduction: xs[c, n] = sum_hw x[n, c, hw].  The reduces
            # are mutually independent (distinct xs columns / xt slots).
            for i, nb in enumerate(TILE_ROWS):
                b = i % NBUF
                vector.wait_ge(sem_x[b], 16 * (i // NBUF + 1))
                n0 = row0[i]
                nc.vector.reduce_sum(
                    xs_t[:, n0 : n0 + nb], xt[b][:, :nb, :], axis=AX.X
                ).then_inc(sem_red, 1)
            # Epilogue: LN -> hardtanh -> LN on [NPC, COUT], stats from PSUM.
            vector.wait_ge(sem_mm, 1)
            hop(nc.vector.reduce_sum(mu1_t[:], ps[:], axis=AX.X))
            hop(nc.vector.tensor_scalar_mul(mu1_t[:], in0=mu1_t[:], scalar1=1.0 / COUT))
            hop(nc.vector.tensor_mul(msq1_t[:], mu1_t[:], mu1_t[:]))
            vector.wait_ge(sem_a, 1)
            # t1 = vs1/COUT - mu1^2   (sqrt adds eps via bias)
            nc.vector.tensor_scalar(
                out=t1_t[:],
                in0=vs1_t[:],
                scalar1=1.0 / COUT,
                scalar2=msq1_t[:],
                op0=ALU.mult,
                op1=ALU.subtract,
            ).then_inc(sem_d, 1)
            vector.wait_ge(sem_a, 2)
            hop(nc.vector.reciprocal(out=sd1_t[:], in_=sd1_t[:]))
            # y1 = (y0 - mu1) * rstd1   [y0 read straight from PSUM]
            hop(
                nc.vector.tensor_scalar(
                    out=y1_t[:],
                    in0=ps[:],
                    scalar1=mu1_t[:],
                    scalar2=sd1_t[:],
                    op0=ALU.subtract,
                    op1=ALU.mult,
                )
            )
            if apply_gb:
                vector.wait_ge(sem_const, n_const)
                hop(nc.vector.tensor_mul(y1_t[:], y1_t[:], g1_t[:]))
                hop(nc.vector.tensor_add(y1_t[:], y1_t[:], b1_t[:]))
            # hardtanh; result feeds both ACT (square) and our own reduce
            nc.vector.tensor_scalar(
                out=y1_t[:],
                in0=y1_t[:],
                scalar1=-1.0,
                scalar2=1.0,
                op0=ALU.max,
                op1=ALU.min,
            ).then_inc(sem_d, 1)
            vector.wait_ge(sem_d, 2)  # self-edge: y1 fully written
            hop(nc.vector.reduce_sum(mu2_t[:], y1_t[:], axis=AX.X))
            hop(nc.vector.tensor_scalar_mul(mu2_t[:], in0=mu2_t[:], scalar1=1.0 / COUT))
            hop(nc.vector.tensor_mul(msq2_t[:], mu2_t[:], mu2_t[:]))
            vector.wait_ge(sem_a, 3)
            nc.vector.tensor_scalar(
                out=t2_t[:],
                in0=vs2_t[:],
                scalar1=1.0 / COUT,
                scalar2=msq2_t[:],
                op0=ALU.mult,
                op1=ALU.subtract,
            ).then_inc(sem_d, 1)
            vector.wait_ge(sem_a, 4)
            hop(nc.vector.reciprocal(out=sd2_t[:], in_=sd2_t[:]))
            if apply_gb:
                hop(
                    nc.vector.tensor_scalar(
                        out=y2_t[:],
                        in0=y1_t[:],
                        scalar1=mu2_t[:],
                        scalar2=sd2_t[:],
                        op0=ALU.subtract,
                        op1=ALU.mult,
                    )
                )
                hop(nc.vector.tensor_mul(y2_t[:], y2_t[:], g2_t[:]))
                nc.vector.tensor_add(y2_t[:], y2_t[:], b2_t[:]).then_inc(sem_y2, 1)
            else:
                nc.vector.tensor_scalar(
                    out=y2_t[:],
                    in0=y1_t[:],
                    scalar1=mu2_t[:],
                    scalar2=sd2_t[:],
                    op0=ALU.subtract,
                    op1=ALU.mult,
                ).then_inc(sem_y2, 1)

        @block.tensor
        def _(tensor):
            tensor.wait_ge(sem_red, NT)
            tensor.wait_ge(sem_const, 32)
            tensor.wait_ge(sem_ms, 2)
            nc.tensor.matmul(ps[:], lhsT=xs_t[:], rhs=wsum_t[:], start=True, stop=False)
            nc.tensor.matmul(
                ps[:], lhsT=ones_t[:], rhs=bias_t[:], start=False, stop=True
            ).then_inc(sem_mm, 1)

    return nc


def _get_nc(apply_gb: bool):
    if apply_gb not in _CACHED_NC:
        _CACHED_NC[apply_gb] = _build_bass(apply_gb)
    return _CACHED_NC[apply_gb]


def _run(inputs, **spmd_kwargs):
    x = np.ascontiguousarray(np.asarray(inputs["x"], dtype=np.float32))
    weight = np.asarray(inputs["weight"], dtype=np.float32)
    bias = np.ascontiguousarray(np.asarray(inputs["bias"], dtype=np.float32))
    g1 = np.ascontiguousarray(np.asarray(inputs["g1"], dtype=np.float32))
    b1 = np.ascontiguousarray(np.asarray(inputs["b1"], dtype=np.float32))
    g2 = np.ascontiguousarray(np.asarray(inputs["g2"], dtype=np.float32))
    b2 = np.ascontiguousarray(np.asarray(inputs["b2"], dtype=np.float32))

    apply_gb = not (
        np.all(g1 == 1.0) and np.all(b1 == 0.0) and np.all(g2 == 1.0) and np.all(b2 == 0.0)
    )
    wsum = np.ascontiguousarray(weight.sum(axis=(2, 3)) / DENOM)  # [CIN, COUT]
    x_sh = x.reshape(NCORES, NPC, CIN, HW)

    nc = _get_nc(apply_gb)
    core_ids = list(range(NCORES))
    in_maps = []
    for i in core_ids:
        m = {"x": np.ascontiguousarray(x_sh[i]), "wsum": wsum, "bias": bias}
        if apply_gb:
            m.update({"g1": g1, "b1": b1, "g2": g2, "b2": b2})
        in_maps.append(m)
    res = run_bass_kernel_spmd(nc, in_maps, core_ids, **spmd_kwargs)
    out = np.concatenate([res.results[i]["out"] for i in core_ids], axis=0)
    return out, res


def kernel(x, weight, bias, g1, b1, g2, b2):
    out, _ = _run(
        {"x": x, "weight": weight, "bias": bias, "g1": g1, "b1": b1, "g2": g2, "b2": b2}
    )
    return out


# revision 18
# speedup vs baseline: 1.1868x; 1.1693x over previous
"""Trainium2 Bass kernel for nn_ModelNew_1580547973039.

Math (see the collapsed reference):
    y0 = (sum_hw x) @ (sum_kk W) / (Hout*Wout) + bias      # [N, Cout]
    y1 = clip(LN(y0)*g1 + b1, -1, 1)
    y2 = LN(y1)*g2 + b2

Sharding: data-parallel over N across 8 cores (32 rows each); params
(weight sums, bias, LN gammas/betas) replicated.  The only heavy work is
streaming x (512 MB total, 64 MB/core) and reducing over the 4096 spatial
positions: [128c, nb*4096hw] tiles, HWDGE DMA double-buffered over NBUF
slots, DVE reduce_sum per tile.  wsum = weight.sum((2,3))/denom is
precomputed on host (4 MB of weight, trivial) and replicated, per the
sharding hint.

Raw Bass (no Tile): this walrus build rejects DMACopy instructions with
more than one sync-wait (one TPB_EVENTS slot), which Tile's scheduler
emits for any slot-recycled streaming loop.  With explicit semaphores the
waits sit on the issuing engine's sequencer instead.  IMPORTANT raw-bass
rule on this stack: engines give NO same-engine ordering of data effects
(DRAIN is a Tile-emitted instruction, absent here) — every same-engine
RAW/WAR edge needs an explicit semaphore hop.

Epilogue uses raw moments (var = E[y^2] - mu^2): ACT's Square+accum runs
concurrently with DVE's mean reduce, both directly from PSUM, then one
fused (y-mu)*rstd tensor_scalar.  When g1/b1/g2/b2 are identity (they are
for this problem's setup_inputs) the gamma/beta ops are compiled out; a
general fallback variant is built otherwise.
"""

import sys
from contextlib import ExitStack

import numpy as np

if "/opt/trn_rl_repo" not in sys.path:
    sys.path.insert(0, "/opt/trn_rl_repo")

import concourse.bass as bass
from concourse import mybir
from concourse.bass_utils import run_bass_kernel_spmd

# Problem shapes (hardcoded; kernel.py must be self-contained).
N, CIN, H, W = 256, 128, 64, 64
COUT = 512
HW = H * W
NCORES = 8
NPC = N // NCORES  # rows of N per core
EPS = 1e-5
STRIDE, KSIZE = 2, 4
HOUT = (H - 1) * STRIDE + KSIZE
WOUT = (W - 1) * STRIDE + KSIZE
DENOM = float(HOUT * WOUT)

# x streaming: 15 tiles of 2 rows (4 MiB) + 2 single-row tail tiles so the
# final reduce on the critical path is half-length.
TILE_ROWS = [2] * 15 + [1] * 2
assert sum(TILE_ROWS) == NPC
NT = len(TILE_ROWS)
NBUF = 5  # [128, 2*4096] f32 slots -> 160 KiB/partition

F32 = mybir.dt.float32
AX = mybir.AxisListType
ALU = mybir.AluOpType
ACTF = mybir.ActivationFunctionType

_CACHED_NC = {}


def _build_bass(apply_gb: bool):
    """Per-core SPMD program: x shard [NPC, CIN, HW] -> out [NPC, COUT]."""
    nc = bass.Bass()

    x_h = nc.declare_dram_parameter("x", [NPC, CIN, HW], F32, isOutput=False)
    wsum_h = nc.declare_dram_parameter("wsum", [CIN, COUT], F32, isOutput=False)
    bias_h = nc.declare_dram_parameter("bias", [COUT], F32, isOutput=False)
    if apply_gb:
        g1_h = nc.declare_dram_parameter("g1", [COUT], F32, isOutput=False)
        b1_h = nc.declare_dram_parameter("b1", [COUT], F32, isOutput=False)
        g2_h = nc.declare_dram_parameter("g2", [COUT], F32, isOutput=False)
        b2_h = nc.declare_dram_parameter("b2", [COUT], F32, isOutput=False)
    out_h = nc.declare_dram_parameter("out", [NPC, COUT], F32, isOutput=True)

    n_const = 96 if apply_gb else 32  # param DMAs x16

    with ExitStack() as ctx:
        sb = lambda name, shape: ctx.enter_context(nc.sbuf_tensor(name, shape, F32))
        xt = [sb(f"xt{b}", [CIN, 2, HW]) for b in range(NBUF)]
        wsum_t = sb("wsum_t", [CIN, COUT])
        bias_t = sb("bias_t", [1, COUT])
        ones_t = sb("ones_t", [1, NPC])
        eps_t = sb("eps_t", [NPC, 1])
        if apply_gb:
            g1_t = sb("g1_t", [NPC, COUT])
            b1_t = sb("b1_t", [NPC, COUT])
            g2_t = sb("g2_t", [NPC, COUT])
            b2_t = sb("b2_t", [NPC, COUT])
        xs_t = sb("xs_t", [CIN, NPC])
        y1_t = sb("y1_t", [NPC, COUT])
        y2_t = sb("y2_t", [NPC, COUT])
        sq_t = sb("sq_t", [NPC, COUT])  # discarded Square output
        mu1_t = sb("mu1_t", [NPC, 1])
        mu2_t = sb("mu2_t", [NPC, 1])
        msq1_t = sb("msq1_t", [NPC, 1])
        msq2_t = sb("msq2_t", [NPC, 1])
        vs1_t = sb("vs1_t", [NPC, 1])
        vs2_t = sb("vs2_t", [NPC, 1])
        t1_t = sb("t1_t", [NPC, 1])
        t2_t = sb("t2_t", [NPC, 1])
        sd1_t = sb("sd1_t", [NPC, 1])
        sd2_t = sb("sd2_t", [NPC, 1])
        ps = ctx.enter_context(nc.psum_tensor("ps", [NPC, COUT], F32))

        sem_x = [ctx.enter_context(nc.semaphore(f"sx{b}")) for b in range(NBUF)]
        sem_const = ctx.enter_context(nc.semaphore("sconst"))
        sem_red = ctx.enter_context(nc.semaphore("sred"))
        sem_mm = ctx.enter_context(nc.semaphore("smm"))
        sem_a = ctx.enter_context(nc.semaphore("sa"))  # ACT -> DVE data
        sem_d = ctx.enter_context(nc.semaphore("sd"))  # DVE -> ACT data
        sem_e = ctx.enter_context(nc.semaphore("se"))  # DVE self-order
        sem_ms = ctx.enter_context(nc.semaphore("sms"))  # memsets done
        sem_y2 = ctx.enter_context(nc.semaphore("sy2"))
        sem_out = ctx.enter_context(nc.semaphore("sout"))
        block = ctx.enter_context(nc.Block())

        row0 = []
        r = 0
        for nb in TILE_ROWS:
            row0.append(r)
            r += nb

        def issue_x_tile(eng, i, nb):
            b = i % NBUF
            if i >= NBUF:
                eng.wait_ge(sem_red, i - NBUF + 1)
            n0 = row0[i]
            eng.dma_start(
                out=xt[b][:, :nb, :],
                in_=x_h[n0 : n0 + nb].rearrange("n c s -> c n s"),
            ).then_inc(sem_x[b], 16)

        @block.sync
        def _(sync):
            # Even x tiles on the SP HWDGE ring.
            for i, nb in enumerate(TILE_ROWS):
                if i % 2 == 0:
                    issue_x_tile(sync, i, nb)
            sync.wait_ge(sem_y2, 1)
            sync.dma_start(out=out_h[:], in_=y2_t[:]).then_inc(sem_out, 16)
            sync.wait_ge(sem_out, 16)

        @block.scalar
        def _(scalar):
            # Replicated params + odd x tiles on the ACT HWDGE ring.
            nc.scalar.dma_start(out=wsum_t[:], in_=wsum_h[:]).then_inc(sem_const, 16)
            nc.scalar.dma_start(out=bias_t[:], in_=bias_h[:][None, :]).then_inc(
                sem_const, 16
            )
            if apply_gb:
                for dst, src in (
                    (g1_t, g1_h),
                    (b1_t, b1_h),
                    (g2_t, g2_h),
                    (b2_t, b2_h),
                ):
                    nc.scalar.dma_start(
                        out=dst[:], in_=src[:][None, :].to_broadcast((NPC, COUT))
                    ).then_inc(sem_const, 16)
            for i, nb in enumerate(TILE_ROWS):
                if i % 2 == 1:
                    issue_x_tile(scalar, i, nb)
            # Variance raw moments + sqrt, interleaved with DVE (see below).
            scalar.wait_ge(sem_ms, 2)
            scalar.wait_ge(sem_mm, 1)
            nc.scalar.activation(
                sq_t[:], ps[:], ACTF.Square, accum_out=vs1_t[:]
            ).then_inc(sem_a, 1)
            scalar.wait_ge(sem_d, 1)
            nc.scalar.activation(sd1_t[:], t1_t[:], ACTF.Sqrt, bias=eps_t[:]).then_inc(
                sem_a, 1
            )
            scalar.wait_ge(sem_d, 2)
            nc.scalar.activation(
                sq_t[:], y1_t[:], ACTF.Square, accum_out=vs2_t[:]
            ).then_inc(sem_a, 1)
            scalar.wait_ge(sem_d, 3)
            nc.scalar.activation(sd2_t[:], t2_t[:], ACTF.Sqrt, bias=eps_t[:]).then_inc(
                sem_a, 1
            )

        @block.vector
        def _(vector):
            ev = 0  # sem_e target tracker

            def hop(inst):
                nonlocal ev
                ev += 1
                inst.then_inc(sem_e, 1)
                vector.wait_ge(sem_e, ev)

            vector.memset(ones_t[:], 1.0).then_inc(sem_ms, 1)
            vector.memset(eps_t[:], EPS).then_inc(sem_ms, 1)
            # Spatial reduction: xs[c, n] = sum_hw x[n, c, hw].  The reduces
            # are mutually independent (distinct xs columns / xt slots).
            for i, nb in enumerate(TILE_ROWS):
                b = i % NBUF
                vector.wait_ge(sem_x[b], 16 * (i // NBUF + 1))
                n0 = row0[i]
                nc.vector.reduce_sum(
                    xs_t[:, n0 : n0 + nb], xt[b][:, :nb, :], axis=AX.X
                ).then_inc(sem_red, 1)
            # Epilogue: LN -> hardtanh -> LN on [NPC, COUT], stats from PSUM.
            vector.wait_ge(sem_mm, 1)
            hop(nc.vector.reduce_sum(mu1_t[:], ps[:], axis=AX.X))
            hop(nc.vector.tensor_scalar_mul(mu1_t[:], in0=mu1_t[:], scalar1=1.0 / COUT))
            hop(nc.vector.tensor_mul(msq1_t[:], mu1_t[:], mu1_t[:]))
            vector.wait_ge(sem_a, 1)
            # t1 = vs1/COUT - mu1^2   (sqrt adds eps via bias)
            nc.vector.tensor_scalar(
                out=t1_t[:],
                in0=vs1_t[:],
                scalar1=1.0 / COUT,
                scalar2=msq1_t[:],
                op0=ALU.mult,
                op1=ALU.subtract,
            ).then_inc(sem_d, 1)
            vector.wait_ge(sem_a, 2)
            hop(nc.vector.reciprocal(out=sd1_t[:], in_=sd1_t[:]))
            # y1 = (y0 - mu1) * rstd1   [y0 read straight from PSUM]
            hop(
                nc.vector.tensor_scalar(
                    out=y1_t[:],
                    in0=ps[:],
                    scalar1=mu1_t[:],
                    scalar2=sd1_t[:],
                    op0=ALU.subtract,
                    op1=ALU.mult,
                )
            )
            if apply_gb:
                vector.wait_ge(sem_const, n_const)
                hop(nc.vector.tensor_mul(y1_t[:], y1_t[:], g1_t[:]))
                hop(nc.vector.tensor_add(y1_t[:], y1_t[:], b1_t[:]))
            # hardtanh; result feeds both ACT (square) and our own reduce
            nc.vector.tensor_scalar(
                out=y1_t[:],
                in0=y1_t[:],
                scalar1=-1.0,
                scalar2=1.0,
                op0=ALU.max,
                op1=ALU.min,
            ).then_inc(sem_d, 1)
            vector.wait_ge(sem_d, 2)  # self-edge: y1 fully written
            hop(nc.vector.reduce_sum(mu2_t[:], y1_t[:], axis=AX.X))
            hop(nc.vector.tensor_scalar_mul(mu2_t[:], in0=mu2_t[:], scalar1=1.0 / COUT))
            hop(nc.vector.tensor_mul(msq2_t[:], mu2_t[:], mu2_t[:]))
            vector.wait_ge(sem_a, 3)
            nc.vector.tensor_scalar(
                out=t2_t[:],
                in0=vs2_t[:],
                scalar1=1.0 / COUT,
                scalar2=msq2_t[:],
                op0=ALU.mult,
                op1=ALU.subtract,
            ).then_inc(sem_d, 1)
            vector.wait_ge(sem_a, 4)
            hop(nc.vector.reciprocal(out=sd2_t[:], in_=sd2_t[:]))
            if apply_gb:
                hop(
                    nc.vector.tensor_scalar(
                        out=y2_t[:],
                        in0=y1_t[:],
                        scalar1=mu2_t[:],
                        scalar2=sd2_t[:],
                        op0=ALU.subtract,
                        op1=ALU.mult,
                    )
                )
                hop(nc.vector.tensor_mul(y2_t[:], y2_t[:], g2_t[:]))
                nc.vector.tensor_add(y2_t[:], y2_t[:], b2_t[:]).then_inc(sem_y2, 1)
            else:
                nc.vector.tensor_scalar(
                    out=y2_t[:],
                    in0=y1_t[:],
                    scalar1=mu2_t[:],
                    scalar2=sd2_t[:],
                    op0=ALU.subtract,
                    op1=ALU.mult,
                ).then_inc(sem_y2, 1)

        @block.tensor
        def _(tensor):
            tensor.wait_ge(sem_red, NT)
            tensor.wait_ge(sem_const, 32)
            tensor.wait_ge(sem_ms, 2)
            nc.tensor.matmul(ps[:], lhsT=xs_t[:], rhs=wsum_t[:], start=True, stop=False)
            nc.tensor.matmul(
                ps[:], lhsT=ones_t[:], rhs=bias_t[:], start=False, stop=True
            ).then_inc(sem_mm, 1)

    return nc


def _get_nc(apply_gb: bool):
    if apply_gb not in _CACHED_NC:
        _CACHED_NC[apply_gb] = _build_bass(apply_gb)
    return _CACHED_NC[apply_gb]


def _run(inputs, **spmd_kwargs):
    x = np.ascontiguousarray(np.asarray(inputs["x"], dtype=np.float32))
    weight = np.asarray(inputs["weight"], dtype=np.float32)
    bias = np.ascontiguousarray(np.asarray(inputs["bias"], dtype=np.float32))
    g1 = np.ascontiguousarray(np.asarray(inputs["g1"], dtype=np.float32))
    b1 = np.ascontiguousarray(np.asarray(inputs["b1"], dtype=np.float32))
    g2 = np.ascontiguousarray(np.asarray(inputs["g2"], dtype=np.float32))
    b2 = np.ascontiguousarray(np.asarray(inputs["b2"], dtype=np.float32))

    apply_gb = not (
        np.all(g1 == 1.0) and np.all(b1 == 0.0) and np.all(g2 == 1.0) and np.all(b2 == 0.0)
    )
    wsum = np.ascontiguousarray(weight.sum(axis=(2, 3)) / DENOM)  # [CIN, COUT]
    x_sh = x.reshape(NCORES, NPC, CIN, HW)

    nc = _get_nc(apply_gb)
    core_ids = list(range(NCORES))
    in_maps = []
    for i in core_ids:
        m = {"x": np.ascontiguousarray(x_sh[i]), "wsum": wsum, "bias": bias}
        if apply_gb:
            m.update({"g1": g1, "b1": b1, "g2": g2, "b2": b2})
        in_maps.append(m)
    res = run_bass_kernel_spmd(nc, in_maps, core_ids, **spmd_kwargs)
    out = np.concatenate([res.results[i]["out"] for i in core_ids], axis=0)
    return out, res


def kernel(x, weight, bias, g1, b1, g2, b2):
    out, _ = _run(
        {"x": x, "weight": weight, "bias": bias, "g1": g1, "b1": b1, "g2": g2, "b2": b2}
    )
    return out
